# revision 24
# baseline (speedup 1.0000x reference)
"""Trainium2 Bass kernel for nn_EnhancedLesionPenaltyLoss.

Loss over pred [16, 1, 128, 128, 128] f32. Pure data parallel: 2 samples per
core across 8 NeuronCores. Each core computes per-partition partial stats with
fused-accumulate DVE/ACT ops; the d-axis runs on the TensorEngine via a
bidiagonal difference matrix with ACT |.|+accumulate on PSUM. The host
finishes the tiny reductions and the loss formula in float64.

Engine split (balanced against the instruction cost model):
  sample 0 convert: DVE  t' = max(s, 0.01)  (fp32->fp16, 2x_2p + free accum)
  sample 1 convert: ACT  t  = relu(s - 0.01) (fp16 out + free accum)
  counts, w-flat pair-max, wrap fixup, boundary sums: DVE
  squares, |d-diff| on PSUM: ACT
  d-diffs, h-pair-max sums: PE matmuls

Self-contained: hardcodes shapes and imports concourse from /opt/trn_rl_repo.
"""

import sys

if "/opt/trn_rl_repo" not in sys.path:
    sys.path.insert(0, "/opt/trn_rl_repo")

import numpy as np

import concourse.bacc as bacc
import concourse.bass as bass
import concourse.mybir as mybir
import concourse.tile as tile
from concourse.bass_utils import run_bass_kernel_spmd

# ---- problem constants ----
B = 16
D = 128
H = 128
W = 128
HW = H * W  # 16384
N_CORES = 8
SAMPLES_PER_CORE = B // N_CORES  # 2
NELEM = D * H * W  # 2097152 per sample
NPAIR = (D - 1) * H * W  # pairs per direction

MIN_T = 0.01
MAX_T = 0.5
TGT_MIN = 0.005
TGT_MAX = 0.03
W_MIN = 15.0
W_MAX = 5.0
W_CONT = 5.0
W_SIZE = 7.0
LESION_T = 0.3

CHUNK = 4096  # free-dim columns per DMA chunk (fp32: 2 MiB per chunk)
NCHUNK = HW // CHUNK  # 4
MM_N = 512
PSUM_COLS = 1536  # d-diff psum tile = 3 banks
ABS_BLOCKS = [1536] * 10 + [1024]  # per-sample |d-diff| block sizes
NABS = len(ABS_BLOCKS)  # 11 abs instructions per sample
MH_FREE = (H - 1) * W  # 16256

# fp16 grid point just above fp16(0.01)=0.0100021362; is_ge on it counts
# exactly the s > 0.01 survivors of the max(s, 0.01) clamp.
C01_GE_THR = 0.010009765625

# DVE stats tile column map (per sample; sample uses columns [s*32, (s+1)*32))
V_CONV = 0  # +NCHUNK: sum of t' (sample 0 only)
V_C01 = V_CONV + NCHUNK
V_C05 = V_C01 + 1
V_MWF = V_C05 + 1  # sum max over flat free-pairs
V_MWRAP = V_MWF + 1
V_MH = V_MWRAP + 1  # h-pair max total (partition 0 only, from PE)
V_H0 = V_MH + 1
V_H127 = V_H0 + 1
V_W0 = V_H127 + 1
V_W127 = V_W0 + 1
V_COLS = 32
assert V_W127 < V_COLS
# ACT stats tile column map (per sample)
A_CONV = 0  # +NCHUNK: sum of t (sample 1 only)
A_SQ = A_CONV + NCHUNK  # +NCHUNK: sum of t^2
A_GD = A_SQ + NCHUNK  # +NABS: sum |d-diff|
A_COLS = 32
assert A_GD + NABS <= A_COLS


def _diff_matrix() -> np.ndarray:
    """lhsT for the PE d-shift: column m = e_{m+1} - e_m (last column zero)."""
    Dm = np.zeros((128, 128), dtype=np.float16)
    for m in range(127):
        Dm[m + 1, m] = 1.0
        Dm[m, m] = -1.0
    return Dm


def _build_program(reps: int = 1):
    nc = bacc.Bacc(
        "TRN2",
        target_bir_lowering=False,
        debug=False,
        enable_asserts=False,
        num_devices=N_CORES,
    )
    x_d = nc.dram_tensor(
        "x", [SAMPLES_PER_CORE, 128, HW], mybir.dt.float32, kind="ExternalInput"
    ).ap()
    dm_d = nc.dram_tensor(
        "dmat", [128, 128], mybir.dt.float16, kind="ExternalInput"
    ).ap()
    stats_d = nc.dram_tensor(
        "stats",
        [2, 128, V_COLS * SAMPLES_PER_CORE],
        mybir.dt.float32,
        kind="ExternalOutput",
    ).ap()

    fp32 = mybir.dt.float32
    fp16 = mybir.dt.float16
    Alu = mybir.AluOpType
    Act = mybir.ActivationFunctionType

    with tile.TileContext(nc) as tc:
        with (
            tc.tile_pool(name="sS", bufs=3) as s_pool,
            tc.tile_pool(name="single", bufs=1) as singles,
            tc.tile_pool(name="psum", bufs=2, space="PSUM") as psum_pool,
            tc.tile_pool(name="psum_acc", bufs=1, space="PSUM") as psum_acc_pool,
        ):
            dmat = singles.tile([128, 128], fp16)
            nc.sync.dma_start(out=dmat[:], in_=dm_d[:])
            ones = singles.tile([128, 1], fp16)
            nc.vector.memset(ones[:], 1.0)
            bias_m001 = singles.tile([128, 1], fp32)
            nc.vector.memset(bias_m001[:], -0.01)
            stats_v = singles.tile([128, V_COLS * SAMPLES_PER_CORE], fp32)
            stats_a = singles.tile([128, A_COLS * SAMPLES_PER_CORE], fp32)

            # Warm-up matmul: folds the dmat-DMA dependency into PE program
            # order.
            warm_ps = psum_pool.tile([128, 128], fp32, name="warm_ps", tag="ps")
            nc.tensor.matmul(warm_ps[:], dmat[:], dmat[:], start=True, stop=True)

            t_tiles = [
                singles.tile([128, HW], fp16, tag=f"t{i}", name=f"t{i}")
                for i in range(2)
            ]
            dve_scr = singles.tile([128, HW], fp16, tag="dscr", name="dscr")
            mh_tile = singles.tile([128, MH_FREE], fp16, tag="mht", name="mht")
            sq_scr = singles.tile([128, CHUNK], fp16, tag="sqscr", name="sqscr")

            for rep_smp in range(reps * SAMPLES_PER_CORE):
                smp = rep_smp % SAMPLES_PER_CORE
                vb = smp * V_COLS
                ab = smp * A_COLS
                tt = t_tiles[smp]
                for c in range(NCHUNK):
                    st = s_pool.tile([128, CHUNK], fp32, name="st", tag="st")
                    nc.sync.dma_start(
                        out=st[:], in_=x_d[smp, :, c * CHUNK : (c + 1) * CHUNK]
                    )
                    tslice = tt[:, c * CHUNK : (c + 1) * CHUNK]
                    if smp == 0:
                        # DVE: t' = max(s, 0.01), accum = sum (2x_2p)
                        nc.vector.tensor_scalar(
                            tslice,
                            st[:],
                            0.01,
                            None,
                            Alu.max,
                            Alu.add,
                            accum_out=stats_v[
                                :, vb + V_CONV + c : vb + V_CONV + c + 1
                            ],
                        )
                    else:
                        # ACT: t = relu(s - 0.01), accum = sum
                        nc.scalar.activation(
                            tslice,
                            st[:],
                            Act.Relu,
                            bias=bias_m001[:],
                            scale=1.0,
                            accum_out=stats_a[
                                :, ab + A_CONV + c : ab + A_CONV + c + 1
                            ],
                        )
                    # ACT: sum of t^2 for this chunk
                    nc.scalar.activation(
                        sq_scr[:],
                        tslice,
                        Act.Square,
                        bias=0.0,
                        scale=1.0,
                        accum_out=stats_a[:, ab + A_SQ + c : ab + A_SQ + c + 1],
                    )

                # PE: d-diffs of t (fp16) into PSUM; ACT in-place abs+accum
                col = 0
                for bi, bcols in enumerate(ABS_BLOCKS):
                    ps = psum_pool.tile([128, PSUM_COLS], fp32, name="ps", tag="ps")
                    for lo in range(0, bcols, MM_N):
                        n = min(MM_N, bcols - lo)
                        nc.tensor.matmul(
                            ps[:, lo : lo + n],
                            dmat[:],
                            tt[:, col + lo : col + lo + n],
                            start=True,
                            stop=True,
                        )
                    gd_col = ab + A_GD + bi
                    nc.scalar.activation(
                        ps[:, :bcols],
                        ps[:, :bcols],
                        Act.Abs,
                        bias=0.0,
                        scale=1.0,
                        accum_out=stats_a[:, gd_col : gd_col + 1],
                    )
                    col += bcols

                # thresholds differ between the max-clamp and relu variants
                thr01 = (C01_GE_THR, Alu.is_ge) if smp == 0 else (0.0, Alu.is_gt)
                thr05 = 0.5 if smp == 0 else 0.49
                nc.vector.tensor_scalar(
                    dve_scr[:],
                    tt[:],
                    thr01[0],
                    None,
                    thr01[1],
                    Alu.add,
                    accum_out=stats_v[:, vb + V_C01 : vb + V_C01 + 1],
                )
                nc.vector.tensor_scalar(
                    dve_scr[:],
                    tt[:],
                    thr05,
                    None,
                    Alu.is_gt,
                    Alu.add,
                    accum_out=stats_v[:, vb + V_C05 : vb + V_C05 + 1],
                )
                t3 = tt[:].rearrange("p (h w) -> p h w", h=H)
                # h-pairs: plain TT max (2x_1p) then PE ones-sum
                nc.vector.tensor_tensor(
                    out=mh_tile[:],
                    in0=t3[:, 1:, :],
                    in1=t3[:, : H - 1, :],
                    op=Alu.max,
                )
                mh_ps = psum_acc_pool.tile([128, MM_N], fp32, name="mhps", tag="mhps")
                nmm = (MH_FREE + MM_N - 1) // MM_N
                for k in range(nmm):
                    lo = k * MM_N
                    hi = min(lo + MM_N, MH_FREE)
                    nc.tensor.matmul(
                        mh_ps[0:1, : hi - lo],
                        ones[:],
                        mh_tile[:, lo:hi],
                        start=(k == 0),
                        stop=(k == nmm - 1),
                    )
                nc.vector.tensor_reduce(
                    stats_v[0:1, vb + V_MH : vb + V_MH + 1],
                    mh_ps[0:1, :],
                    axis=mybir.AxisListType.X,
                    op=Alu.add,
                )
                # flat free-pairs (w-pairs plus h-wrap pairs), fused accum
                nc.vector.scalar_tensor_tensor(
                    out=dve_scr[:, : HW - 1],
                    in0=tt[:, 1:HW],
                    scalar=0.0,
                    in1=tt[:, : HW - 1],
                    op0=Alu.bypass,
                    op1=Alu.max,
                    accum_out=stats_v[:, vb + V_MWF : vb + V_MWF + 1],
                )
                # wrap pairs: (h,127) -> (h+1,0)
                wrap_a = t3[:, 1:, 0:1].rearrange("p h one -> p (h one)")
                wrap_b = t3[:, : H - 1, W - 1 : W].rearrange("p h one -> p (h one)")
                nc.vector.scalar_tensor_tensor(
                    out=dve_scr[:, : H - 1],
                    in0=wrap_a,
                    scalar=0.0,
                    in1=wrap_b,
                    op0=Alu.bypass,
                    op1=Alu.max,
                    accum_out=stats_v[:, vb + V_MWRAP : vb + V_MWRAP + 1],
                )
                # thin boundary sums on t
                for col, view in (
                    (V_H0, t3[:, 0:1, :].rearrange("p one w -> p (one w)")),
                    (V_H127, t3[:, H - 1 : H, :].rearrange("p one w -> p (one w)")),
                    (V_W0, t3[:, :, 0:1].rearrange("p h one -> p (h one)")),
                    (V_W127, t3[:, :, W - 1 : W].rearrange("p h one -> p (h one)")),
                ):
                    nc.vector.tensor_reduce(
                        stats_v[:, vb + col : vb + col + 1],
                        view,
                        axis=mybir.AxisListType.X,
                        op=Alu.add,
                    )

            nc.sync.dma_start(out=stats_d[0], in_=stats_v[:])
            nc.sync.dma_start(out=stats_d[1], in_=stats_a[:])
    nc.compile()
    return nc


_NC_CACHE = {}


def _get_program(reps: int = 1):
    if reps not in _NC_CACHE:
        _NC_CACHE[reps] = _build_program(reps)
    return _NC_CACHE[reps]


def _host_reduce(stats_all: np.ndarray) -> np.float32:
    """stats_all: [n_cores, 2, 128, 64] fp32 -> scalar loss (float32)."""
    total = 0.0
    for i in range(B):
        core = i // SAMPLES_PER_CORE
        smp = i % SAMPLES_PER_CORE
        sv = stats_all[core][0][:, smp * V_COLS : (smp + 1) * V_COLS].astype(
            np.float64
        )
        sa = stats_all[core][1][:, smp * A_COLS : (smp + 1) * A_COLS].astype(
            np.float64
        )
        if smp == 0:
            conv_rows = sv[:, V_CONV : V_CONV + NCHUNK].sum(axis=1)
        else:
            conv_rows = sa[:, A_CONV : A_CONV + NCHUNK].sum(axis=1)
        sum_tp = conv_rows.sum()
        c01 = sv[:, V_C01].sum()
        c05 = sv[:, V_C05].sum()
        mh = sv[0, V_MH]
        mwf = sv[:, V_MWF].sum()
        mwrap = sv[:, V_MWRAP].sum()
        ch0 = sv[:, V_H0].sum()
        ch127 = sv[:, V_H127].sum()
        cw0 = sv[:, V_W0].sum()
        cw127 = sv[:, V_W127].sum()
        sum_tp2 = sa[:, A_SQ : A_SQ + NCHUNK].sum()
        gd = sa[:, A_GD : A_GD + NABS].sum()

        act = c01 / NELEM
        high = c05 / NELEM
        loss = max(TGT_MIN - act, 0.0) * W_MIN
        loss += max(high - TGT_MAX, 0.0) * W_MAX

        # continuity: sum |adjacent difference| per direction
        g_h = 2.0 * mh - 2.0 * sum_tp + ch0 + ch127
        g_w = 2.0 * (mwf - mwrap) - 2.0 * sum_tp + cw0 + cw127
        g_d = gd
        avg_grad = (g_d + g_h + g_w) / (3.0 * NPAIR)
        has_lesion = c05 > 0.0  # any(s > 0.5) implies any(s > 0.3)
        if has_lesion:
            loss += min(avg_grad, 1.0) * W_CONT

        # size-variance penalty (masked stats)
        cnt = c01
        if smp == 0:  # t' = max(s, 0.01)
            s1 = sum_tp - MIN_T * (NELEM - c01)
            s2 = sum_tp2 - MIN_T * MIN_T * (NELEM - c01)
        else:  # t = relu(s - 0.01)
            s1 = sum_tp + MIN_T * c01
            s2 = sum_tp2 + 2.0 * MIN_T * sum_tp + MIN_T * MIN_T * c01
        cnt_safe = max(cnt, 1.0)
        m = s1 / cnt_safe
        sq = s2 - 2.0 * m * s1 + m * m * cnt
        gate = (act > 0.001) and (cnt > 1.0)
        if gate:
            var = sq / max(cnt - 1.0, 1.0)
            std = np.sqrt(max(var, 0.0))
            rel_std = std / (m + 1e-6)
            pen = np.exp(-5.0 * rel_std)
            loss += pen * W_SIZE

        total += loss
    return np.float32(total / B)


def _run_cores(in_maps, trace=False, reps=1):
    nc = _get_program(reps)
    return run_bass_kernel_spmd(
        nc, in_maps, core_ids=list(range(N_CORES)), trace=trace
    )


def _make_in_maps(pred: np.ndarray):
    dm = _diff_matrix()
    in_maps = []
    for c in range(N_CORES):
        shard = np.ascontiguousarray(
            pred[c * SAMPLES_PER_CORE : (c + 1) * SAMPLES_PER_CORE, 0].reshape(
                SAMPLES_PER_CORE, 128, HW
            ),
            dtype=np.float32,
        )
        in_maps.append({"x": shard, "dmat": dm})
    return in_maps


def kernel(pred: np.ndarray) -> np.ndarray:
    pred = np.asarray(pred, dtype=np.float32)
    assert pred.shape == (B, 1, D, H, W), pred.shape
    res = _run_cores(_make_in_maps(pred), trace=False)
    stats_all = np.stack([r["stats"] for r in res.results])
    return _host_reduce(stats_all)


# revision 29
# speedup vs baseline: 1.1661x; 1.1661x over previous
"""Trainium2 Bass kernel for nn_EnhancedLesionPenaltyLoss.

Loss over pred [16, 1, 128, 128, 128] f32. Pure data parallel: 2 samples per
core across 8 NeuronCores. Each core computes per-partition partial stats with
fused-accumulate DVE/ACT ops; the d-axis runs on the TensorEngine via a
bidiagonal difference matrix with ACT |.|+accumulate on PSUM. The host
finishes the tiny reductions and the loss formula in float64.

Engine split (balanced against the instruction cost model):
  sample 0 convert: DVE  t' = max(s, 0.01)  (fp32->fp16, 2x_2p + free accum)
  sample 1 convert: ACT  t  = relu(s - 0.01) (fp16 out + free accum)
  counts, w-flat pair-max, wrap fixup, boundary sums: DVE
  squares, |d-diff| on PSUM: ACT
  d-diffs, h-pair-max sums: PE matmuls

Self-contained: hardcodes shapes and imports concourse from /opt/trn_rl_repo.
"""

import sys

if "/opt/trn_rl_repo" not in sys.path:
    sys.path.insert(0, "/opt/trn_rl_repo")

import numpy as np

import concourse.bacc as bacc
import concourse.bass as bass
import concourse.mybir as mybir
import concourse.tile as tile
from concourse.bass_utils import run_bass_kernel_spmd

# ---- problem constants ----
B = 16
D = 128
H = 128
W = 128
HW = H * W  # 16384
N_CORES = 8
SAMPLES_PER_CORE = B // N_CORES  # 2
NELEM = D * H * W  # 2097152 per sample
NPAIR = (D - 1) * H * W  # pairs per direction

MIN_T = 0.01
MAX_T = 0.5
TGT_MIN = 0.005
TGT_MAX = 0.03
W_MIN = 15.0
W_MAX = 5.0
W_CONT = 5.0
W_SIZE = 7.0
LESION_T = 0.3

CHUNK = 4096  # free-dim columns per DMA chunk (fp32: 2 MiB per chunk)
NCHUNK = HW // CHUNK  # 4
MM_N = 512
PSUM_COLS = 1536  # d-diff psum tile = 3 banks
CHUNK_ABS_BLOCKS = [1536, 1536, 1024]  # per-chunk |d-diff| block sizes
NABS = len(CHUNK_ABS_BLOCKS) * NCHUNK  # 12 abs instructions per sample
MH_FREE = (H - 1) * W  # 16256

# fp16 grid point just above fp16(0.01)=0.0100021362; is_ge on it counts
# exactly the s > 0.01 survivors of the max(s, 0.01) clamp.
C01_GE_THR = 0.010009765625

# DVE stats tile column map (per sample; sample uses columns [s*32, (s+1)*32))
V_CONV = 0  # +NCHUNK: sum of t' (sample 0 only)
V_C01 = V_CONV + NCHUNK
V_C05 = V_C01 + 1
V_MWF = V_C05 + 1  # sum max over flat free-pairs
V_MWRAP = V_MWF + 1
V_MH = V_MWRAP + 1  # h-pair max total (partition 0 only, from PE)
V_H0 = V_MH + 1
V_H127 = V_H0 + 1
V_W0 = V_H127 + 1
V_W127 = V_W0 + 1
V_COLS = 32
assert V_W127 < V_COLS
# ACT stats tile column map (per sample)
A_CONV = 0  # +NCHUNK: sum of t (sample 1 only)
A_SQ = A_CONV + NCHUNK  # +NCHUNK: sum of t^2
A_GD = A_SQ + NCHUNK  # +NABS: sum |d-diff|
A_COLS = 32
assert A_GD + NABS <= A_COLS


def _diff_matrix() -> np.ndarray:
    """lhsT for the PE d-shift: column m = e_{m+1} - e_m (last column zero)."""
    Dm = np.zeros((128, 128), dtype=np.float32)
    for m in range(127):
        Dm[m + 1, m] = 1.0
        Dm[m, m] = -1.0
    return Dm


def _build_program(reps: int = 1):
    nc = bacc.Bacc(
        "TRN2",
        target_bir_lowering=False,
        debug=False,
        enable_asserts=False,
        num_devices=N_CORES,
    )
    x_d = nc.dram_tensor(
        "x", [SAMPLES_PER_CORE, 128, HW], mybir.dt.float32, kind="ExternalInput"
    ).ap()
    dm_d = nc.dram_tensor(
        "dmat", [128, 128], mybir.dt.float32, kind="ExternalInput"
    ).ap()
    stats_d = nc.dram_tensor(
        "stats",
        [2, 128, V_COLS * SAMPLES_PER_CORE],
        mybir.dt.float32,
        kind="ExternalOutput",
    ).ap()

    fp32 = mybir.dt.float32
    fp16 = mybir.dt.float16
    Alu = mybir.AluOpType
    Act = mybir.ActivationFunctionType

    with tile.TileContext(nc) as tc:
        with (
            tc.tile_pool(name="sS", bufs=3) as s_pool,
            tc.tile_pool(name="single", bufs=1) as singles,
            tc.tile_pool(name="psum", bufs=2, space="PSUM") as psum_pool,
            tc.tile_pool(name="psum_acc", bufs=1, space="PSUM") as psum_acc_pool,
        ):
            dmat = singles.tile([128, 128], fp32)
            nc.sync.dma_start(out=dmat[:], in_=dm_d[:])
            ones = singles.tile([128, 1], fp16)
            nc.vector.memset(ones[:], 1.0)
            bias_m001 = singles.tile([128, 1], fp32)
            nc.vector.memset(bias_m001[:], -0.01)
            stats_v = singles.tile([128, V_COLS * SAMPLES_PER_CORE], fp32)
            stats_a = singles.tile([128, A_COLS * SAMPLES_PER_CORE], fp32)

            # Warm-up matmul: folds the dmat-DMA dependency into PE program
            # order.
            warm_ps = psum_pool.tile([128, 128], fp32, name="warm_ps", tag="ps")
            nc.tensor.matmul(warm_ps[:], dmat[:], dmat[:], start=True, stop=True)

            t_tiles = [
                singles.tile([128, HW], fp16, tag=f"t{i}", name=f"t{i}")
                for i in range(2)
            ]
            dve_scr = singles.tile([128, HW], fp16, tag="dscr", name="dscr")
            mh_tile = singles.tile([128, MH_FREE], fp16, tag="mht", name="mht")
            sq_scr = singles.tile([128, CHUNK], fp16, tag="sqscr", name="sqscr")

            for rep_smp in range(reps * SAMPLES_PER_CORE):
                smp = rep_smp % SAMPLES_PER_CORE
                vb = smp * V_COLS
                ab = smp * A_COLS
                tt = t_tiles[smp]
                for c in range(NCHUNK):
                    st = s_pool.tile([128, CHUNK], fp32, name="st", tag="st")
                    nc.sync.dma_start(
                        out=st[:], in_=x_d[smp, :, c * CHUNK : (c + 1) * CHUNK]
                    )
                    tslice = tt[:, c * CHUNK : (c + 1) * CHUNK]
                    if smp == 0:
                        # DVE: t' = max(s, 0.01), accum = sum (2x_2p)
                        nc.vector.tensor_scalar(
                            tslice,
                            st[:],
                            0.01,
                            None,
                            Alu.max,
                            Alu.add,
                            accum_out=stats_v[
                                :, vb + V_CONV + c : vb + V_CONV + c + 1
                            ],
                        )
                    else:
                        # ACT: t = relu(s - 0.01), accum = sum
                        nc.scalar.activation(
                            tslice,
                            st[:],
                            Act.Relu,
                            bias=bias_m001[:],
                            scale=1.0,
                            accum_out=stats_a[
                                :, ab + A_CONV + c : ab + A_CONV + c + 1
                            ],
                        )
                    # ACT: sum of t^2 for this chunk
                    nc.scalar.activation(
                        sq_scr[:],
                        tslice,
                        Act.Square,
                        bias=0.0,
                        scale=1.0,
                        accum_out=stats_a[:, ab + A_SQ + c : ab + A_SQ + c + 1],
                    )
                    # PE: exact d-diffs of raw fp32 s into PSUM (depends only
                    # on the chunk DMA, so it overlaps the converts); ACT
                    # in-place abs+accum
                    col = 0
                    for bi, bcols in enumerate(CHUNK_ABS_BLOCKS):
                        ps = psum_pool.tile(
                            [128, PSUM_COLS], fp32, name="ps", tag="ps"
                        )
                        for lo in range(0, bcols, MM_N):
                            n = min(MM_N, bcols - lo)
                            nc.tensor.matmul(
                                ps[:, lo : lo + n],
                                dmat[:],
                                st[:, col + lo : col + lo + n],
                                start=True,
                                stop=True,
                            )
                        gd_col = ab + A_GD + c * len(CHUNK_ABS_BLOCKS) + bi
                        nc.scalar.activation(
                            ps[:, :bcols],
                            ps[:, :bcols],
                            Act.Abs,
                            bias=0.0,
                            scale=1.0,
                            accum_out=stats_a[:, gd_col : gd_col + 1],
                        )
                        col += bcols

                # thresholds differ between the max-clamp and relu variants
                thr01 = (C01_GE_THR, Alu.is_ge) if smp == 0 else (0.0, Alu.is_gt)
                thr05 = 0.5 if smp == 0 else 0.49
                nc.vector.tensor_scalar(
                    dve_scr[:],
                    tt[:],
                    thr01[0],
                    None,
                    thr01[1],
                    Alu.add,
                    accum_out=stats_v[:, vb + V_C01 : vb + V_C01 + 1],
                )
                nc.vector.tensor_scalar(
                    dve_scr[:],
                    tt[:],
                    thr05,
                    None,
                    Alu.is_gt,
                    Alu.add,
                    accum_out=stats_v[:, vb + V_C05 : vb + V_C05 + 1],
                )
                t3 = tt[:].rearrange("p (h w) -> p h w", h=H)
                # h-pairs: plain TT max (2x_1p) then PE ones-sum
                nc.vector.tensor_tensor(
                    out=mh_tile[:],
                    in0=t3[:, 1:, :],
                    in1=t3[:, : H - 1, :],
                    op=Alu.max,
                )
                mh_ps = psum_acc_pool.tile([128, MM_N], fp32, name="mhps", tag="mhps")
                nmm = (MH_FREE + MM_N - 1) // MM_N
                for k in range(nmm):
                    lo = k * MM_N
                    hi = min(lo + MM_N, MH_FREE)
                    nc.tensor.matmul(
                        mh_ps[0:1, : hi - lo],
                        ones[:],
                        mh_tile[:, lo:hi],
                        start=(k == 0),
                        stop=(k == nmm - 1),
                    )
                nc.vector.tensor_reduce(
                    stats_v[0:1, vb + V_MH : vb + V_MH + 1],
                    mh_ps[0:1, :],
                    axis=mybir.AxisListType.X,
                    op=Alu.add,
                )
                # flat free-pairs (w-pairs plus h-wrap pairs), fused accum
                nc.vector.scalar_tensor_tensor(
                    out=dve_scr[:, : HW - 1],
                    in0=tt[:, 1:HW],
                    scalar=0.0,
                    in1=tt[:, : HW - 1],
                    op0=Alu.bypass,
                    op1=Alu.max,
                    accum_out=stats_v[:, vb + V_MWF : vb + V_MWF + 1],
                )
                # wrap pairs: (h,127) -> (h+1,0)
                wrap_a = t3[:, 1:, 0:1].rearrange("p h one -> p (h one)")
                wrap_b = t3[:, : H - 1, W - 1 : W].rearrange("p h one -> p (h one)")
                nc.vector.scalar_tensor_tensor(
                    out=dve_scr[:, : H - 1],
                    in0=wrap_a,
                    scalar=0.0,
                    in1=wrap_b,
                    op0=Alu.bypass,
                    op1=Alu.max,
                    accum_out=stats_v[:, vb + V_MWRAP : vb + V_MWRAP + 1],
                )
                # thin boundary sums on t
                for col, view in (
                    (V_H0, t3[:, 0:1, :].rearrange("p one w -> p (one w)")),
                    (V_H127, t3[:, H - 1 : H, :].rearrange("p one w -> p (one w)")),
                    (V_W0, t3[:, :, 0:1].rearrange("p h one -> p (h one)")),
                    (V_W127, t3[:, :, W - 1 : W].rearrange("p h one -> p (h one)")),
                ):
                    nc.vector.tensor_reduce(
                        stats_v[:, vb + col : vb + col + 1],
                        view,
                        axis=mybir.AxisListType.X,
                        op=Alu.add,
                    )

            nc.sync.dma_start(out=stats_d[0], in_=stats_v[:])
            nc.sync.dma_start(out=stats_d[1], in_=stats_a[:])
    nc.compile()
    return nc


_NC_CACHE = {}


def _get_program(reps: int = 1):
    if reps not in _NC_CACHE:
        _NC_CACHE[reps] = _build_program(reps)
    return _NC_CACHE[reps]


def _host_reduce(stats_all: np.ndarray) -> np.float32:
    """stats_all: [n_cores, 2, 128, 64] fp32 -> scalar loss (float32)."""
    total = 0.0
    for i in range(B):
        core = i // SAMPLES_PER_CORE
        smp = i % SAMPLES_PER_CORE
        sv = stats_all[core][0][:, smp * V_COLS : (smp + 1) * V_COLS].astype(
            np.float64
        )
        sa = stats_all[core][1][:, smp * A_COLS : (smp + 1) * A_COLS].astype(
            np.float64
        )
        if smp == 0:
            conv_rows = sv[:, V_CONV : V_CONV + NCHUNK].sum(axis=1)
        else:
            conv_rows = sa[:, A_CONV : A_CONV + NCHUNK].sum(axis=1)
        sum_tp = conv_rows.sum()
        c01 = sv[:, V_C01].sum()
        c05 = sv[:, V_C05].sum()
        mh = sv[0, V_MH]
        mwf = sv[:, V_MWF].sum()
        mwrap = sv[:, V_MWRAP].sum()
        ch0 = sv[:, V_H0].sum()
        ch127 = sv[:, V_H127].sum()
        cw0 = sv[:, V_W0].sum()
        cw127 = sv[:, V_W127].sum()
        sum_tp2 = sa[:, A_SQ : A_SQ + NCHUNK].sum()
        gd = sa[:, A_GD : A_GD + NABS].sum()

        act = c01 / NELEM
        high = c05 / NELEM
        loss = max(TGT_MIN - act, 0.0) * W_MIN
        loss += max(high - TGT_MAX, 0.0) * W_MAX

        # continuity: sum |adjacent difference| per direction
        g_h = 2.0 * mh - 2.0 * sum_tp + ch0 + ch127
        g_w = 2.0 * (mwf - mwrap) - 2.0 * sum_tp + cw0 + cw127
        g_d = gd
        avg_grad = (g_d + g_h + g_w) / (3.0 * NPAIR)
        has_lesion = c05 > 0.0  # any(s > 0.5) implies any(s > 0.3)
        if has_lesion:
            loss += min(avg_grad, 1.0) * W_CONT

        # size-variance penalty (masked stats)
        cnt = c01
        if smp == 0:  # t' = max(s, 0.01)
            s1 = sum_tp - MIN_T * (NELEM - c01)
            s2 = sum_tp2 - MIN_T * MIN_T * (NELEM - c01)
        else:  # t = relu(s - 0.01)
            s1 = sum_tp + MIN_T * c01
            s2 = sum_tp2 + 2.0 * MIN_T * sum_tp + MIN_T * MIN_T * c01
        cnt_safe = max(cnt, 1.0)
        m = s1 / cnt_safe
        sq = s2 - 2.0 * m * s1 + m * m * cnt
        gate = (act > 0.001) and (cnt > 1.0)
        if gate:
            var = sq / max(cnt - 1.0, 1.0)
            std = np.sqrt(max(var, 0.0))
            rel_std = std / (m + 1e-6)
            pen = np.exp(-5.0 * rel_std)
            loss += pen * W_SIZE

        total += loss
    return np.float32(total / B)


def _run_cores(in_maps, trace=False, reps=1):
    nc = _get_program(reps)
    return run_bass_kernel_spmd(
        nc, in_maps, core_ids=list(range(N_CORES)), trace=trace
    )


def _make_in_maps(pred: np.ndarray):
    dm = _diff_matrix()
    in_maps = []
    for c in range(N_CORES):
        shard = np.ascontiguousarray(
            pred[c * SAMPLES_PER_CORE : (c + 1) * SAMPLES_PER_CORE, 0].reshape(
                SAMPLES_PER_CORE, 128, HW
            ),
            dtype=np.float32,
        )
        in_maps.append({"x": shard, "dmat": dm})
    return in_maps


def kernel(pred: np.ndarray) -> np.ndarray:
    pred = np.asarray(pred, dtype=np.float32)
    assert pred.shape == (B, 1, D, H, W), pred.shape
    res = _run_cores(_make_in_maps(pred), trace=False)
    stats_all = np.stack([r["stats"] for r in res.results])
    return _host_reduce(stats_all)
